# revision 4
# baseline (speedup 1.0000x reference)
"""Trainium2 Bass kernel for the PINN-style loss problem.

Math: a 6-layer tanh MLP u(x,t) (2->50x5->1) is evaluated with forward-mode
jets (u, u_x, u_t, u_xxx) at N=10000 points. The per-param loss
  loss_p = mean_n (u_t + a_p*u*u_x + b_p*u_xxx + c_p*u_x)^2
collapses to a quadratic form q^T G q with q = (a,b,c,1)/sqrt(N) and G the
4x4 Gram matrix of g_n = [u*u_x, u_xxx, u_x, u_t].

Sharding: x is split into 8 slices of 1250 points (one per NeuronCore);
each core builds its partial Gram, an AllReduce sums them, then each core
evaluates the quadratic form for its 625-row slice of para.

Device layout: points are packed 2-per-partition-block (block-diagonal
weights, K=100), free dim 640 per block (block0: 640 real points,
block1: 610 real + 30 zero-padded, masked out before the Gram matmul).
"""

import os
import sys
import numpy as np

for _p in ("/opt/trn_rl_repo",):
    if os.path.isdir(_p) and _p not in sys.path:
        sys.path.append(_p)

import concourse.bass as bass
import concourse.bacc as bacc
import concourse.mybir as mybir
import concourse.tile as tile
from concourse import bass_utils

F32 = mybir.dt.float32
F32R = mybir.dt.float32r
AF = mybir.ActivationFunctionType
ALU = mybir.AluOpType

NCORES = 8
NPTS = 10000
NPC = NPTS // NCORES       # 1250 points per core
PPC = 5000 // NCORES       # 625 para rows per core
FD = 640                   # free dim per block (block0 full, block1 padded)
B1 = NPC - FD              # 610 real points in block1
HB = 100                   # 2 blocks x 50 hidden units
CHUNKS = ((0, 512), (512, 128))      # matmul free-dim chunks (psum bank limit)
PCH = ((0, 512), (512, PPC - 512))   # para free-dim chunks

SDT = F32R                 # tower stream/weight dtype (f32r: 1 cyc/row matmul)

# const blob A (per-core): [4, CA] = h0(640) | w1t(100) | paraT4(625)
CA_H0 = 0
CA_W1T = 640
CA_PARA = 740
CA = 740 + PPC             # 1365
# const blob B (shared): [128, CB] = wb(400) | w6p(2) | vecs(10) | b6bc(2)
CB_WB = 0
CB_W6P = 400
CB_VECS = 402
CB_B6BC = 412
CB = 414


def _mm(nc, out, lhsT, rhs, start=True, stop=True):
    nc.tensor.matmul(out, lhsT, rhs, start=start, stop=stop)


def _mm_chunks(nc, out_tile, lhsT, rhs_tile, chunks=CHUNKS):
    for off, w in chunks:
        _mm(nc, out_tile[:, off:off + w], lhsT, rhs_tile[:, off:off + w])


def build_program(stage="full"):
    nc = bacc.Bacc("TRN2", target_bir_lowering=False, debug=False)

    cstA_d = nc.dram_tensor("cstA", [4, CA], SDT, kind="ExternalInput")
    cstB_d = nc.dram_tensor("cstB", [128, CB], SDT, kind="ExternalInput")
    if stage == "tower":
        loss_d = nc.dram_tensor("dbg", [HB, FD], F32, kind="ExternalOutput")
    elif stage in ("l6", "cc"):
        loss_d = nc.dram_tensor("dbg", [4, 4], F32, kind="ExternalOutput")
    else:
        loss_d = nc.dram_tensor("loss", [1, PPC], F32, kind="ExternalOutput")

    with tile.TileContext(nc) as tc:
        _body(tc, nc, cstA_d, cstB_d, loss_d, stage=stage)
    nc.compile()
    return nc


def _body(tc, nc, cstA_d, cstB_d, loss_d, stage="full"):
    import contextlib

    ctx = contextlib.ExitStack()
    with ctx:
        cpool = ctx.enter_context(tc.tile_pool(name="const", bufs=1))
        spool = ctx.enter_context(tc.tile_pool(name="streams", bufs=2))
        tpool = ctx.enter_context(tc.tile_pool(name="trans", bufs=2))
        dpool = ctx.enter_context(tc.tile_pool(name="dram", bufs=1, space="DRAM"))

        # ---- load constants (2 blob DMAs on different queues) ----
        cstA = cpool.tile([4, CA], SDT, tag="cstA")
        cstB = cpool.tile([128, CB], SDT, tag="cstB")
        nc.sync.dma_start(cstA[:], cstA_d[:])
        nc.scalar.dma_start(cstB[:], cstB_d[:])

        h0 = cstA[:, CA_H0:CA_H0 + FD]
        w1t = cstA[:, CA_W1T:CA_W1T + HB]
        paraT4 = cstA[:, CA_PARA:CA_PARA + PPC].bitcast(F32)
        wb = cstB[0:HB, CB_WB:CB_WB + 400]
        w6p = cstB[0:HB, CB_W6P:CB_W6P + 2]
        vecs = cstB[0:HB, CB_VECS:CB_VECS + 10].bitcast(F32)
        b6bc = cstB[:, CB_B6BC:CB_B6BC + 2].bitcast(F32)

        ones4 = cpool.tile([4, 1], F32, tag="ones4")
        nc.vector.memset(ones4[:], 1.0)

        cx = vecs[:, 0:1]
        ct = vecs[:, 1:2]
        cx2 = vecs[:, 2:3]
        cx3 = vecs[:, 3:4]

        def bb(layer):  # bias vector for layer 1..5
            return vecs[:, 3 + layer:4 + layer]

        neg2 = vecs[:, 9:10]

        v = nc.vector
        s = nc.scalar
        g = nc.gpsimd

        a5 = ax5 = at5 = axxx5 = None

        with tc.tile_pool(name="ztw", bufs=3, space="PSUM") as zpool:
            # ---------- layer 1 ----------
            z = zpool.tile([HB, FD], F32, tag="ztw")
            _mm_chunks(nc, z, w1t, h0)
            a = spool.tile([HB, FD], SDT, tag="a")
            s.activation(a[:], z[:], AF.Tanh, bias=bb(1))
            asq = tpool.tile([HB, FD], F32, tag="asq")
            s.activation(asq[:], a[:], AF.Square)
            f1 = tpool.tile([HB, FD], F32, tag="f1")
            s.activation(f1[:], asq[:], AF.Identity, scale=-1.0, bias=1.0)
            h6 = tpool.tile([HB, FD], F32, tag="h6")
            s.activation(h6[:], asq[:], AF.Identity, scale=6.0, bias=neg2)
            ax = spool.tile([HB, FD], SDT, tag="ax")
            v.tensor_scalar(ax[:], f1[:], cx, None, ALU.mult)
            at = spool.tile([HB, FD], SDT, tag="at")
            v.tensor_scalar(at[:], f1[:], ct, None, ALU.mult)
            af1 = tpool.tile([HB, FD], F32, tag="p1")
            v.tensor_tensor(af1[:], a[:], f1[:], ALU.mult)
            axx = spool.tile([HB, FD], SDT, tag="axx")
            v.tensor_scalar(axx[:], af1[:], cx2, -2.0, ALU.mult, ALU.mult)
            f3 = tpool.tile([HB, FD], F32, tag="p2")
            g.tensor_tensor(f3[:], f1[:], h6[:], ALU.mult)
            axxx = spool.tile([HB, FD], SDT, tag="axxx")
            v.tensor_scalar(axxx[:], f3[:], cx3, None, ALU.mult)

            # ---------- layers 2..5 ----------
            for layer in range(2, 6):
                W = wb[:, 100 * (layer - 2):100 * (layer - 1)]
                last = layer == 5

                z = zpool.tile([HB, FD], F32, tag="ztw")
                _mm_chunks(nc, z, W, a)
                a_n = spool.tile([HB, FD], SDT, tag="a")
                s.activation(a_n[:], z[:], AF.Tanh, bias=bb(layer))

                zt = zpool.tile([HB, FD], F32, tag="ztw")
                _mm_chunks(nc, zt, W, at)
                asq = tpool.tile([HB, FD], F32, tag="asq")
                s.activation(asq[:], a_n[:], AF.Square)
                f1 = tpool.tile([HB, FD], F32, tag="f1")
                s.activation(f1[:], asq[:], AF.Identity, scale=-1.0, bias=1.0)
                at_n = spool.tile([HB, FD], SDT, tag="at")
                v.tensor_tensor(at_n[:], f1[:], zt[:], ALU.mult)

                zx = zpool.tile([HB, FD], F32, tag="ztw")
                _mm_chunks(nc, zx, W, ax)
                h6 = tpool.tile([HB, FD], F32, tag="h6")
                s.activation(h6[:], asq[:], AF.Identity, scale=6.0, bias=neg2)
                ax_n = spool.tile([HB, FD], SDT, tag="ax")
                v.tensor_tensor(ax_n[:], f1[:], zx[:], ALU.mult)
                w2 = tpool.tile([HB, FD], F32, tag="w2")
                s.activation(w2[:], zx[:], AF.Square)
                P = tpool.tile([HB, FD], F32, tag="p1")
                v.tensor_tensor(P[:], a_n[:], zx[:], ALU.mult)
                zx3 = tpool.tile([HB, FD], F32, tag="zx3")
                v.tensor_tensor(zx3[:], w2[:], zx[:], ALU.mult)

                zxx = zpool.tile([HB, FD], F32, tag="ztw")
                _mm_chunks(nc, zxx, W, axx)
                if not last:
                    gt = tpool.tile([HB, FD], F32, tag="g")
                    g.tensor_tensor(gt[:], a_n[:], w2[:], ALU.mult)
                    inner = tpool.tile([HB, FD], F32, tag="inner")
                    v.scalar_tensor_tensor(inner[:], gt[:], -2.0, zxx[:],
                                           ALU.mult, ALU.add)
                m = tpool.tile([HB, FD], F32, tag="p2")
                v.tensor_tensor(m[:], P[:], zxx[:], ALU.mult)
                if not last:
                    axx_n = spool.tile([HB, FD], SDT, tag="axx")
                    g.tensor_tensor(axx_n[:], f1[:], inner[:], ALU.mult)

                zxxx = zpool.tile([HB, FD], F32, tag="ztw")
                _mm_chunks(nc, zxxx, W, axxx)
                i3a = tpool.tile([HB, FD], F32, tag="i3a")
                v.scalar_tensor_tensor(i3a[:], m[:], -6.0, zxxx[:],
                                       ALU.mult, ALU.add)
                n_t = tpool.tile([HB, FD], F32, tag="n")
                g.tensor_tensor(n_t[:], h6[:], zx3[:], ALU.mult)
                i3 = tpool.tile([HB, FD], F32, tag="i3")
                g.tensor_tensor(i3[:], i3a[:], n_t[:], ALU.add)
                axxx_n = spool.tile([HB, FD], SDT, tag="axxx")
                v.tensor_tensor(axxx_n[:], f1[:], i3[:], ALU.mult)

                a, at, ax, axxx = a_n, at_n, ax_n, axxx_n
                if not last:
                    axx = axx_n

            a5, ax5, at5, axxx5 = a, ax, at, axxx

        if stage == "tower":
            nc.sync.dma_start(loss_d[:], axxx5[:].bitcast(F32))
            return

        # ---------- layer 6 + Gram ----------
        # chunk tiles: [128 points, 10] cols: s-major pairs (b0,b1) for
        # s=0 uux, 1 uxxx, 2 ux, 3 ut; cols 8:10 = u.
        with tc.tile_pool(name="l6c", bufs=2, space="PSUM") as l6p, \
             tc.tile_pool(name="psmall", bufs=1, space="PSUM") as pps:
            G = pps.tile([4, 4], F32, tag="gram")
            for c in range(5):
                lo = 128 * c
                ch = l6p.tile([128, 10], F32, tag="l6c")
                _mm(nc, ch[:, 8:10], a5[:, lo:lo + 128], w6p)
                _mm(nc, ch[:, 2:4], axxx5[:, lo:lo + 128], w6p)
                _mm(nc, ch[:, 4:6], ax5[:, lo:lo + 128], w6p)
                _mm(nc, ch[:, 6:8], at5[:, lo:lo + 128], w6p)
                chS = tpool.tile([128, 10], F32, tag="l6s")
                v.tensor_copy(chS[:, 2:10], ch[:, 2:10])
                # uux = (u + b6) * ux
                v.scalar_tensor_tensor(chS[:, 0:2], chS[:, 8:10], b6bc[:128, 0:1],
                                       chS[:, 4:6], ALU.add, ALU.mult)
                chv = chS[:, 0:8].rearrange("p (s b) -> p b s", b=2, s=4)
                if c == 4 and B1 < FD:
                    # zero the padded block1 points before the Gram matmul
                    v.tensor_scalar(chv[:, 1, :], chv[:, 1, :], b6bc[:128, 1:2],
                                    None, ALU.mult)
                for b in range(2):
                    st = c == 0 and b == 0
                    sp = c == 4 and b == 1
                    nc.tensor.matmul(G[:], chv[:, b, :], chv[:, b, :],
                                     start=st, stop=sp)

            gS = cpool.tile([4, 4], F32, tag="gS")
            v.tensor_copy(gS[:], G[:])

            if stage == "l6":
                nc.sync.dma_start(loss_d[:], gS[:])
                return

            # ---------- AllReduce the Gram ----------
            gin = dpool.tile([4, 4], F32, tag="gin")
            gout = dpool.tile([4, 4], F32, tag="gout")
            nc.gpsimd.dma_start(gin[:], gS[:])
            nc.gpsimd.collective_compute(
                "AllReduce",
                ALU.add,
                replica_groups=[list(range(NCORES))],
                ins=[gin.opt()],
                outs=[gout.opt()],
            )
            G4 = cpool.tile([4, 4], F32, tag="G4")
            nc.gpsimd.dma_start(G4[:], gout[:])

            if stage == "cc":
                nc.sync.dma_start(loss_d[:], G4[:])
                return

            # ---------- para quadratic form: loss = q^T G q, q = para4/sqrt(N) ----------
            PS = pps.tile([4, PPC], F32, tag="PS")
            for off, w in PCH:
                _mm(nc, PS[:, off:off + w], G4[:], paraT4[:, off:off + w])
            H4 = cpool.tile([4, PPC], F32, tag="H4")
            v.tensor_tensor(H4[:], PS[:], paraT4[:], ALU.mult)
            LP = pps.tile([1, PPC], F32, tag="LP")
            for off, w in PCH:
                _mm(nc, LP[:, off:off + w], ones4[:], H4[:, off:off + w])
            lossS = cpool.tile([1, PPC], F32, tag="lossS")
            s.activation(lossS[:], LP[:], AF.Copy)
            nc.gpsimd.dma_start(loss_d[:], lossS[:])


def prep_inputs(x, para, W1, b1, W2, b2, W3, b3, W4, b4, W5, b5, W6, b6):
    """Full inputs -> list of per-core input dicts (host-side shard/layout)."""
    f = np.float32
    x = np.asarray(x, f)
    para = np.asarray(para, f)
    Ws = [np.asarray(W, f) for W in (W1, W2, W3, W4, W5, W6)]
    bs = [np.asarray(b, f) for b in (b1, b2, b3, b4, b5, b6)]

    cstB = np.zeros((128, CB), f)
    for i in range(4):
        W = Ws[i + 1]
        cstB[0:50, CB_WB + 100 * i:CB_WB + 100 * i + 50] = W.T
        cstB[50:100, CB_WB + 100 * i + 50:CB_WB + 100 * i + 100] = W.T
    cstB[0:50, CB_W6P] = Ws[5][0]
    cstB[50:100, CB_W6P + 1] = Ws[5][0]
    vecs = cstB[:, CB_VECS:CB_VECS + 10]
    vecs[0:HB, 9] = -2.0
    cx = Ws[0][:, 0]
    ct = Ws[0][:, 1]
    for half in (slice(0, 50), slice(50, 100)):
        vecs[half, 0] = cx
        vecs[half, 1] = ct
        vecs[half, 2] = cx * cx
        vecs[half, 3] = cx * cx * cx
        for l in range(5):
            vecs[half, 4 + l] = bs[l]
    cstB[:, CB_B6BC] = bs[5][0]
    cstB[:, CB_B6BC + 1] = 1.0
    cstB[B1 - 512:, CB_B6BC + 1] = 0.0

    w1t = np.zeros((4, HB), f)
    w1t[0:2, 0:50] = Ws[0].T
    w1t[2:4, 50:100] = Ws[0].T

    rsqn = 1.0 / np.sqrt(np.float32(NPTS))
    maps = []
    for c in range(NCORES):
        sl = x[c * NPC:(c + 1) * NPC]
        cstA = np.zeros((4, CA), f)
        cstA[0, CA_H0:CA_H0 + FD] = sl[0:FD, 0]
        cstA[1, CA_H0:CA_H0 + FD] = sl[0:FD, 1]
        cstA[2, CA_H0:CA_H0 + B1] = sl[FD:NPC, 0]
        cstA[3, CA_H0:CA_H0 + B1] = sl[FD:NPC, 1]
        cstA[:, CA_W1T:CA_W1T + HB] = w1t
        pslice = para[c * PPC:(c + 1) * PPC]
        cstA[0:3, CA_PARA:CA_PARA + PPC] = pslice.T * rsqn
        cstA[3, CA_PARA:CA_PARA + PPC] = rsqn
        maps.append({"cstA": cstA, "cstB": cstB})
    return maps


_NC_CACHE = {}


def get_program():
    if "nc" not in _NC_CACHE:
        _NC_CACHE["nc"] = build_program()
    return _NC_CACHE["nc"]


def kernel(x, para, W1, b1, W2, b2, W3, b3, W4, b4, W5, b5, W6, b6):
    maps = prep_inputs(x, para, W1, b1, W2, b2, W3, b3, W4, b4, W5, b5, W6, b6)
    nc = get_program()
    res = bass_utils.run_bass_kernel_spmd(nc, maps, list(range(NCORES)))
    out = np.concatenate([res.results[c]["loss"].reshape(-1) for c in range(NCORES)])
    return out.astype(np.float32)


# revision 7
# speedup vs baseline: 1.1131x; 1.1131x over previous
"""Trainium2 Bass kernel for the PINN-style loss problem.

Math: a 6-layer tanh MLP u(x,t) (2->50x5->1) is evaluated with forward-mode
jets (u, u_x, u_t, u_xxx) at N=10000 points. The per-param loss
  loss_p = mean_n (u_t + a_p*u*u_x + b_p*u_xxx + c_p*u_x)^2
collapses to a quadratic form q^T G q with q = (a,b,c,1)/sqrt(N) and G the
4x4 Gram matrix of g_n = [u*u_x, u_xxx, u_x, u_t].

Sharding: x is split into 8 slices of 1250 points (one per NeuronCore);
each core builds its partial Gram, an AllReduce sums them, then each core
evaluates the quadratic form for its 625-row slice of para.

Device layout: points are packed 2-per-partition-block (block-diagonal
weights, K=100), free dim 640 per block (block0: 640 real points,
block1: 610 real + 30 zero-padded, masked out before the Gram matmul).

Engine plan: tower streams are bf16 so DVE tensor_scalar/scalar_tensor_tensor
ops hit the 4x perf mode; Act handles tanh/squares/PSUM->SBUF copies; the
Pool (gpsimd) queue carries ONLY the collective + its bounce DMAs so the
AllReduce mesh is never stuck behind compute.
"""

import os
import sys
import numpy as np

for _p in ("/opt/trn_rl_repo",):
    if os.path.isdir(_p) and _p not in sys.path:
        sys.path.append(_p)

import concourse.bass as bass
import concourse.bacc as bacc
import concourse.mybir as mybir
import concourse.tile as tile
from concourse import bass_utils

F32 = mybir.dt.float32
F32R = mybir.dt.float32r
BF16 = mybir.dt.float16
AF = mybir.ActivationFunctionType
ALU = mybir.AluOpType

NCORES = 8
NPTS = 10000
NPC = NPTS // NCORES       # 1250 points per core
PPC = 5000 // NCORES       # 625 para rows per core
FD = 640                   # free dim per block (block0 full, block1 padded)
B1 = NPC - FD              # 610 real points in block1
HB = 100                   # 2 blocks x 50 hidden units
CHUNKS = ((0, 512), (512, 128))      # matmul free-dim chunks (psum bank limit)
PCH = ((0, 512), (512, PPC - 512))   # para free-dim chunks

SDT = F32R                 # layer-1 dtype (input points at full precision)
BDT = BF16                 # tower stream/weight dtype (2-byte: DVE 4x mode)
WARM_CC = True             # early dummy collective on the otherwise-idle pool queue

# const blob A (per-core, f32r): h0(640) | w1t(100) | paraT4(625)
CA_H0 = 0
CA_W1T = 640
CA_PARA = 740
CA = 740 + PPC             # 1365
# const blob B (shared): bf16 [128, CBW=400+2] wb|w6p  +  f32 [128, CBF] vecs|b6bc
CBW_WB = 0
CBW_W6P = 400
CBW = 402
CBF_VECS = 0
CBF_B6BC = 10
CBF = 12


def _mm(nc, out, lhsT, rhs, start=True, stop=True):
    nc.tensor.matmul(out, lhsT, rhs, start=start, stop=stop)


def _mm_chunks(nc, out_tile, lhsT, rhs_tile, chunks=CHUNKS):
    for off, w in chunks:
        _mm(nc, out_tile[:, off:off + w], lhsT, rhs_tile[:, off:off + w])


def build_program(stage="full"):
    nc = bacc.Bacc("TRN2", target_bir_lowering=False, debug=False)

    cstA_d = nc.dram_tensor("cstA", [4, CA], SDT, kind="ExternalInput")
    cstW_d = nc.dram_tensor("cstW", [128, CBW], BDT, kind="ExternalInput")
    cstF_d = nc.dram_tensor("cstF", [128, CBF], F32, kind="ExternalInput")
    if stage == "tower":
        loss_d = nc.dram_tensor("dbg", [HB, FD], F32, kind="ExternalOutput")
    elif stage in ("l6", "cc"):
        loss_d = nc.dram_tensor("dbg", [4, 4], F32, kind="ExternalOutput")
    else:
        loss_d = nc.dram_tensor("loss", [1, PPC], F32, kind="ExternalOutput")

    with tile.TileContext(nc) as tc:
        _body(tc, nc, cstA_d, cstW_d, cstF_d, loss_d, stage=stage)
    nc.compile()
    return nc


def _body(tc, nc, cstA_d, cstW_d, cstF_d, loss_d, stage="full"):
    import contextlib

    ctx = contextlib.ExitStack()
    with ctx:
        cpool = ctx.enter_context(tc.tile_pool(name="const", bufs=1))
        spool = ctx.enter_context(tc.tile_pool(name="streams", bufs=2))
        tpool = ctx.enter_context(tc.tile_pool(name="trans", bufs=2))
        dpool = ctx.enter_context(tc.tile_pool(name="dram", bufs=1, space="DRAM"))

        v = nc.vector
        s = nc.scalar
        g = nc.gpsimd

        # ---- warm collective: first instructions on the pool queue ----
        if WARM_CC and stage not in ("tower",):
            wsrc = cpool.tile([1, 1], F32, tag="wsrc")
            g.memset(wsrc[:], 1.0)
            win = dpool.tile([1, 1], F32, tag="win")
            wout = dpool.tile([1, 1], F32, tag="wout")
            g.dma_start(win[:], wsrc[:])
            nc.gpsimd.collective_compute(
                "AllReduce", ALU.add,
                replica_groups=[list(range(NCORES))],
                ins=[win.opt()], outs=[wout.opt()],
            )

        # ---- load constants (3 blob DMAs on different queues) ----
        cstA = cpool.tile([4, CA], SDT, tag="cstA")
        cstW = cpool.tile([128, CBW], BDT, tag="cstW")
        cstF = cpool.tile([128, CBF], F32, tag="cstF")
        nc.sync.dma_start(cstA[:], cstA_d[:])
        nc.scalar.dma_start(cstW[:], cstW_d[:])
        nc.sync.dma_start(cstF[:], cstF_d[:])

        h0 = cstA[:, CA_H0:CA_H0 + FD]
        w1t = cstA[:, CA_W1T:CA_W1T + HB]
        paraT4 = cstA[:, CA_PARA:CA_PARA + PPC].bitcast(F32)
        wb = cstW[0:HB, CBW_WB:CBW_WB + 400]
        w6p = cstW[0:HB, CBW_W6P:CBW_W6P + 2]
        vecs = cstF[0:HB, CBF_VECS:CBF_VECS + 10]
        b6bc = cstF[:, CBF_B6BC:CBF_B6BC + 2]

        ones4 = cpool.tile([4, 1], F32, tag="ones4")
        v.memset(ones4[:], 1.0)

        cx = vecs[:, 0:1]
        ct = vecs[:, 1:2]
        cx2 = vecs[:, 2:3]
        cx3 = vecs[:, 3:4]

        def bb(layer):  # bias vector for layer 1..5
            return vecs[:, 3 + layer:4 + layer]

        neg2 = vecs[:, 9:10]

        def TS(out, in0, s1, s2, op1, op2=None):
            v.tensor_scalar(out, in0, s1, s2, op1, op2 or ALU.mult)

        a5 = ax5 = at5 = axxx5 = None

        with tc.tile_pool(name="ztw", bufs=3, space="PSUM") as zpool:
            # ---------- layer 1 (f32r matmul, bf16 outputs) ----------
            z = zpool.tile([HB, FD], F32, tag="ztw")
            _mm_chunks(nc, z, w1t, h0)
            a = spool.tile([HB, FD], BDT, tag="a")
            s.activation(a[:], z[:], AF.Tanh, bias=bb(1))
            gsq = tpool.tile([HB, FD], BDT, tag="gsq")
            s.activation(gsq[:], a[:], AF.Square)
            f1 = tpool.tile([HB, FD], BDT, tag="f1")
            v.tensor_scalar(f1[:], gsq[:], -1.0, 1.0, ALU.mult, ALU.add)
            h6 = tpool.tile([HB, FD], BDT, tag="h6")
            v.tensor_scalar(h6[:], gsq[:], 6.0, -2.0, ALU.mult, ALU.add)
            ax = spool.tile([HB, FD], BDT, tag="ax")
            v.tensor_scalar(ax[:], f1[:], cx, None, ALU.mult)
            at = spool.tile([HB, FD], BDT, tag="at")
            v.tensor_scalar(at[:], f1[:], ct, None, ALU.mult)
            af1 = tpool.tile([HB, FD], BDT, tag="p1")
            v.scalar_tensor_tensor(af1[:], a[:], 1.0, f1[:], ALU.mult, ALU.mult)
            axx = spool.tile([HB, FD], BDT, tag="axx")
            v.tensor_scalar(axx[:], af1[:], cx2, -2.0, ALU.mult, ALU.mult)
            f3 = tpool.tile([HB, FD], BDT, tag="p2")
            v.scalar_tensor_tensor(f3[:], f1[:], 1.0, h6[:], ALU.mult, ALU.mult)
            axxx = spool.tile([HB, FD], BDT, tag="axxx")
            v.tensor_scalar(axxx[:], f3[:], cx3, None, ALU.mult)

            # ---------- layers 2..5 ----------
            for layer in range(2, 6):
                W = wb[:, 100 * (layer - 2):100 * (layer - 1)]
                last = layer == 5

                z = zpool.tile([HB, FD], F32, tag="ztw")
                _mm_chunks(nc, z, W, a)
                a_n = spool.tile([HB, FD], BDT, tag="a")
                s.activation(a_n[:], z[:], AF.Tanh, bias=bb(layer))

                zt = zpool.tile([HB, FD], F32, tag="ztw")
                _mm_chunks(nc, zt, W, at)
                gsq = tpool.tile([HB, FD], BDT, tag="gsq")
                s.activation(gsq[:], a_n[:], AF.Square)
                f1 = tpool.tile([HB, FD], BDT, tag="f1")
                v.tensor_scalar(f1[:], gsq[:], -1.0, 1.0, ALU.mult, ALU.add)
                at_n = spool.tile([HB, FD], BDT, tag="at")
                v.scalar_tensor_tensor(at_n[:], f1[:], 1.0, zt[:], ALU.mult,
                                       ALU.mult)

                zx = zpool.tile([HB, FD], F32, tag="ztw")
                _mm_chunks(nc, zx, W, ax)
                zxC = tpool.tile([HB, FD], BDT, tag="zxC")
                s.activation(zxC[:], zx[:], AF.Copy)
                w2 = tpool.tile([HB, FD], BDT, tag="w2")
                s.activation(w2[:], zx[:], AF.Square)
                h6 = tpool.tile([HB, FD], BDT, tag="h6")
                v.tensor_scalar(h6[:], gsq[:], 6.0, -2.0, ALU.mult, ALU.add)
                ax_n = spool.tile([HB, FD], BDT, tag="ax")
                v.scalar_tensor_tensor(ax_n[:], f1[:], 1.0, zxC[:], ALU.mult,
                                       ALU.mult)
                P = tpool.tile([HB, FD], BDT, tag="p1")
                v.scalar_tensor_tensor(P[:], a_n[:], 1.0, zxC[:], ALU.mult,
                                       ALU.mult)
                zx3 = tpool.tile([HB, FD], BDT, tag="zx3")
                v.scalar_tensor_tensor(zx3[:], w2[:], 1.0, zxC[:], ALU.mult,
                                       ALU.mult)

                zxx = zpool.tile([HB, FD], F32, tag="ztw")
                _mm_chunks(nc, zxx, W, axx)
                zxxC = tpool.tile([HB, FD], BDT, tag="zxxC")
                s.activation(zxxC[:], zxx[:], AF.Copy)
                if not last:
                    q = tpool.tile([HB, FD], BDT, tag="q")
                    v.scalar_tensor_tensor(q[:], P[:], 1.0, zxC[:], ALU.mult,
                                           ALU.mult)
                    inner = tpool.tile([HB, FD], BDT, tag="inner")
                    v.scalar_tensor_tensor(inner[:], q[:], -2.0, zxxC[:],
                                           ALU.mult, ALU.add)
                    axx_n = spool.tile([HB, FD], BDT, tag="axx")
                    v.scalar_tensor_tensor(axx_n[:], f1[:], 1.0, inner[:],
                                           ALU.mult, ALU.mult)
                m = tpool.tile([HB, FD], BDT, tag="p2")
                v.scalar_tensor_tensor(m[:], P[:], 1.0, zxxC[:], ALU.mult,
                                       ALU.mult)

                zxxx = zpool.tile([HB, FD], F32, tag="ztw")
                _mm_chunks(nc, zxxx, W, axxx)
                i3a = tpool.tile([HB, FD], BDT, tag="i3a")
                v.scalar_tensor_tensor(i3a[:], m[:], -6.0, zxxx[:],
                                       ALU.mult, ALU.add)
                nt = tpool.tile([HB, FD], BDT, tag="n")
                v.scalar_tensor_tensor(nt[:], h6[:], 1.0, zx3[:], ALU.mult,
                                       ALU.mult)
                i3 = tpool.tile([HB, FD], BDT, tag="i3")
                v.scalar_tensor_tensor(i3[:], nt[:], 1.0, i3a[:], ALU.mult,
                                       ALU.add)
                axxx_n = spool.tile([HB, FD], BDT, tag="axxx")
                v.scalar_tensor_tensor(axxx_n[:], f1[:], 1.0, i3[:], ALU.mult,
                                       ALU.mult)

                a, at, ax, axxx = a_n, at_n, ax_n, axxx_n
                if not last:
                    axx = axx_n

            a5, ax5, at5, axxx5 = a, ax, at, axxx

        if stage == "tower":
            nc.sync.dma_start(loss_d[:], axxx5[:].bitcast(mybir.dt.uint16))
            return

        # ---------- layer 6 + Gram ----------
        # chunk tiles: [128 points, 10] cols: s-major pairs (b0,b1) for
        # s=0 uux, 1 uxxx, 2 ux, 3 ut; cols 8:10 = u.
        with tc.tile_pool(name="l6c", bufs=2, space="PSUM") as l6p, \
             tc.tile_pool(name="psmall", bufs=1, space="PSUM") as pps:
            G = pps.tile([4, 4], F32, tag="gram")
            for c in range(5):
                lo = 128 * c
                ch = l6p.tile([128, 10], F32, tag="l6c")
                _mm(nc, ch[:, 8:10], a5[:, lo:lo + 128], w6p)
                _mm(nc, ch[:, 2:4], axxx5[:, lo:lo + 128], w6p)
                _mm(nc, ch[:, 4:6], ax5[:, lo:lo + 128], w6p)
                _mm(nc, ch[:, 6:8], at5[:, lo:lo + 128], w6p)
                chS = tpool.tile([128, 10], BDT, tag="l6s")
                v.tensor_copy(chS[:, 2:10], ch[:, 2:10])
                # uux = (u + b6) * ux
                v.scalar_tensor_tensor(chS[:, 0:2], chS[:, 8:10], b6bc[:128, 0:1],
                                       chS[:, 4:6], ALU.add, ALU.mult)
                chv = chS[:, 0:8].rearrange("p (s b) -> p b s", b=2, s=4)
                if c == 4 and B1 < FD:
                    # zero the padded block1 points before the Gram matmul
                    v.tensor_scalar(chv[:, 1, :], chv[:, 1, :], b6bc[:128, 1:2],
                                    None, ALU.mult)
                for b in range(2):
                    st = c == 0 and b == 0
                    sp = c == 4 and b == 1
                    nc.tensor.matmul(G[:], chv[:, b, :], chv[:, b, :],
                                     start=st, stop=sp)

            gS = cpool.tile([4, 4], F32, tag="gS")
            v.tensor_copy(gS[:], G[:])

            if stage == "l6":
                nc.sync.dma_start(loss_d[:], gS[:])
                return

            # ---------- AllReduce the Gram ----------
            gin = dpool.tile([4, 4], F32, tag="gin")
            gout = dpool.tile([4, 4], F32, tag="gout")
            nc.gpsimd.dma_start(gin[:], gS[:])
            nc.gpsimd.collective_compute(
                "AllReduce",
                ALU.add,
                replica_groups=[list(range(NCORES))],
                ins=[gin.opt()],
                outs=[gout.opt()],
            )
            G4 = cpool.tile([4, 4], F32, tag="G4")
            nc.gpsimd.dma_start(G4[:], gout[:])

            if stage == "cc":
                nc.sync.dma_start(loss_d[:], G4[:])
                return

            # ---------- para quadratic form: loss = q^T G q, q = para4/sqrt(N) ----------
            PS = pps.tile([4, PPC], F32, tag="PS")
            for off, w in PCH:
                _mm(nc, PS[:, off:off + w], G4[:], paraT4[:, off:off + w])
            H4 = cpool.tile([4, PPC], F32, tag="H4")
            v.tensor_tensor(H4[:], PS[:], paraT4[:], ALU.mult)
            LP = pps.tile([1, PPC], F32, tag="LP")
            for off, w in PCH:
                _mm(nc, LP[:, off:off + w], ones4[:], H4[:, off:off + w])
            lossS = cpool.tile([1, PPC], F32, tag="lossS")
            s.activation(lossS[:], LP[:], AF.Copy)
            nc.gpsimd.dma_start(loss_d[:], lossS[:])


def prep_inputs(x, para, W1, b1, W2, b2, W3, b3, W4, b4, W5, b5, W6, b6):
    """Full inputs -> list of per-core input dicts (host-side shard/layout)."""
    import ml_dtypes
    f = np.float32
    bf = np.float16
    x = np.asarray(x, f)
    para = np.asarray(para, f)
    Ws = [np.asarray(W, f) for W in (W1, W2, W3, W4, W5, W6)]
    bs = [np.asarray(b, f) for b in (b1, b2, b3, b4, b5, b6)]

    cstW = np.zeros((128, CBW), bf)
    for i in range(4):
        W = Ws[i + 1]
        cstW[0:50, CBW_WB + 100 * i:CBW_WB + 100 * i + 50] = W.T
        cstW[50:100, CBW_WB + 100 * i + 50:CBW_WB + 100 * i + 100] = W.T
    cstW[0:50, CBW_W6P] = Ws[5][0]
    cstW[50:100, CBW_W6P + 1] = Ws[5][0]

    cstF = np.zeros((128, CBF), f)
    vecs = cstF[:, CBF_VECS:CBF_VECS + 10]
    vecs[0:HB, 9] = -2.0
    cx = Ws[0][:, 0]
    ct = Ws[0][:, 1]
    for half in (slice(0, 50), slice(50, 100)):
        vecs[half, 0] = cx
        vecs[half, 1] = ct
        vecs[half, 2] = cx * cx
        vecs[half, 3] = cx * cx * cx
        for l in range(5):
            vecs[half, 4 + l] = bs[l]
    cstF[:, CBF_B6BC] = bs[5][0]
    cstF[:, CBF_B6BC + 1] = 1.0
    cstF[B1 - 512:, CBF_B6BC + 1] = 0.0

    w1t = np.zeros((4, HB), f)
    w1t[0:2, 0:50] = Ws[0].T
    w1t[2:4, 50:100] = Ws[0].T

    rsqn = 1.0 / np.sqrt(np.float32(NPTS))
    maps = []
    for c in range(NCORES):
        sl = x[c * NPC:(c + 1) * NPC]
        cstA = np.zeros((4, CA), f)
        cstA[0, CA_H0:CA_H0 + FD] = sl[0:FD, 0]
        cstA[1, CA_H0:CA_H0 + FD] = sl[0:FD, 1]
        cstA[2, CA_H0:CA_H0 + B1] = sl[FD:NPC, 0]
        cstA[3, CA_H0:CA_H0 + B1] = sl[FD:NPC, 1]
        cstA[:, CA_W1T:CA_W1T + HB] = w1t
        pslice = para[c * PPC:(c + 1) * PPC]
        cstA[0:3, CA_PARA:CA_PARA + PPC] = pslice.T * rsqn
        cstA[3, CA_PARA:CA_PARA + PPC] = rsqn
        maps.append({"cstA": cstA, "cstW": cstW, "cstF": cstF})
    return maps


_NC_CACHE = {}


def get_program():
    if "nc" not in _NC_CACHE:
        _NC_CACHE["nc"] = build_program()
    return _NC_CACHE["nc"]


def kernel(x, para, W1, b1, W2, b2, W3, b3, W4, b4, W5, b5, W6, b6):
    maps = prep_inputs(x, para, W1, b1, W2, b2, W3, b3, W4, b4, W5, b5, W6, b6)
    nc = get_program()
    res = bass_utils.run_bass_kernel_spmd(nc, maps, list(range(NCORES)))
    out = np.concatenate([res.results[c]["loss"].reshape(-1) for c in range(NCORES)])
    return out.astype(np.float32)
